# revision 12
# baseline (speedup 1.0000x reference)
"""Trainium2 kernel for nn_MultiHeadClassifier.

Math: out[i] = W[task_labels[i]] @ x[i] + b[task_labels[i]]
  x [262144, 1024] f32, task_labels [262144] int, W [8, 32, 1024], b [8, 32]

Strategy (8 NeuronCores, data-parallel over batch), v3:
  - The problem is HBM-bound: the only large tensor is x. v1 streamed x as
    f32 (128 MiB/core) and computed all 8 heads on the PE, selecting via a
    one-hot mask (8x the needed matmul work). v2+ halves the traffic and
    cuts PE work 8x:
      * x is cast to bf16 on the host (tolerance is 2e-2; bf16 adds ~2.6e-3).
      * Rows are routed on the host: each core's 32768 rows are placed
        into 8 static 4096-row blocks by task id. The device schedule is
        fully static: rows in block t use W[t]. No masks, no padding.
      * Block overflow (a task with >4096 rows on one core; ~24 rows
        expected per block) is computed on the host in numpy and patched
        into the output. Underfull blocks hold zero rows (harmless).
  - Device inner loop: per 512-row chunk, 8 accumulating matmuls with the
    block's W as the stationary operand ([128k, 32], N=512 moving rows
    from the [ki, rows]-transposed x), psum [32, 512] -> DVE copy/cast to
    bf16 -> per-superblock DMA out as [32, rows].
  - v4: first superblock streams as 4x512-row DMAs (the HWDGE descriptor
    ramp delays the first bytes ~3us; small first transfers start the
    stream sooner) and the last superblock as 8x256-row DMAs with
    per-piece output DMAs so the pipeline tail overlaps the final bytes'
    arrival; consts load first so the PE warms up early.
  - Host: inverse permutation, bias add, f32 cast.
"""

import sys

sys.path.insert(0, "/opt/trn_rl_repo")

import numpy as np
import ml_dtypes

import concourse.bass as bass
import concourse.tile as tile
from concourse import bacc, mybir
from concourse import bass_utils

B, D, C, T = 262144, 1024, 32, 8
NCORES = 8
N = B // NCORES  # 32768 rows per core
P = 128
KO = D // P  # 8 contraction tiles
BLK = N // T  # 4096 rows per task block (static capacity)
SB = 2048  # rows per superblock (one x DMA = 4 MB)
NSB = N // SB  # 16 superblocks per core
CHUNK = 512  # rows per psum accumulation group
NCH = SB // CHUNK  # chunks per superblock
NMID = NSB - 2  # whole superblocks between the split first/last ones
ROW0 = SB  # first row covered by whole superblocks
NROW1 = (NSB - 1) * SB  # first row of the split tail
TCH = 256  # tail piece rows
NTCH = SB // TCH  # tail pieces

# set by test harness to collect a profile; harness-invoked kernel() keeps it off
TRACE = False
LAST_RESULTS = None


def _build():
    f32 = mybir.dt.float32
    bf16 = mybir.dt.bfloat16

    nc = bacc.Bacc("TRN2", debug=False, num_devices=NCORES)
    # xt[sb, ki, ko, r]: rows already routed into task blocks; 32 KB
    # contiguous per partition per superblock -> near-peak DMA efficiency.
    xt_d = nc.dram_tensor("xt", [NMID, P, KO, SB], bf16, kind="ExternalInput")
    # first superblock, chunk-major: starts the stream sooner
    xtf_d = nc.dram_tensor(
        "xtf", [NCH, P, KO, CHUNK], bf16, kind="ExternalInput"
    )
    # last superblock, piece-major so the tail streams at 256-row grain
    xtl_d = nc.dram_tensor(
        "xtl", [NTCH, P, KO, TCH], bf16, kind="ExternalInput"
    )
    # wall[ki, t, ko, c] = W[t, c, ko*128+ki] (lhsT layout, all 8 heads)
    wall_d = nc.dram_tensor("wall", [P, T, KO, C], bf16, kind="ExternalInput")
    out_d = nc.dram_tensor("out", [C, N], bf16, kind="ExternalOutput")

    with tile.TileContext(nc) as tc:
        with (
            tc.tile_pool(name="sbuf", bufs=1) as sbuf,
            tc.tile_pool(name="xpool", bufs=3) as xpool,
            tc.tile_pool(name="fpool", bufs=4) as fpool,
            tc.tile_pool(name="psum", bufs=8, space="PSUM") as psum,
        ):
            # consts first on the ACT ring (small, so the PE warmup isn't
            # stuck behind x superblocks); the SP ring is a pure x stream
            wall = sbuf.tile([P, T, KO, C], bf16)
            nc.scalar.dma_start(wall[:], wall_d[:])

            # prologue x pieces in flight immediately
            xtfs = []
            for st in range(NCH):
                xtf = fpool.tile([P, KO, CHUNK], bf16, tag="xtf")
                nc.sync.dma_start(xtf[:], xtf_d[st])
                xtfs.append(xtf)

            # Engine warmups: one instruction per engine that observes the
            # const DMA lane, so steady-state instructions carry at most
            # one semaphore wait each.
            scratch = psum.tile([C, CHUNK], f32, tag="y")
            nc.tensor.matmul(
                scratch[:2, :2], wall[:2, 0, 0, :2], wall[:2, 0, 0, :2],
                start=True, stop=True,
            )
            dve_scr = sbuf.tile([P, C], bf16, tag="dve_scr")
            nc.vector.tensor_copy(dve_scr[:], wall[:, 0, 0, :])

            def chunk_group(y, xap, t, out_slice, n):
                for ko in range(KO):
                    nc.tensor.matmul(
                        y[:, :n],
                        wall[:, t, ko, :],
                        xap(ko),
                        start=(ko == 0),
                        stop=(ko == KO - 1),
                    )
                nc.vector.tensor_copy(out_slice, y[:, :n])

            # prologue compute (rows [0, SB) are all task 0)
            out_f = xpool.tile([C, SB], bf16, tag="out_sb")
            for st in range(NCH):
                y = psum.tile([C, CHUNK], f32, tag="y")
                chunk_group(
                    y,
                    lambda ko: xtfs[st][:, ko, :],
                    (st * CHUNK) // BLK,
                    out_f[:, st * CHUNK : (st + 1) * CHUNK],
                    CHUNK,
                )
            nc.scalar.dma_start(out_d[:, :SB], out_f[:])

            for i in range(NMID):
                r0 = ROW0 + i * SB
                xts = xpool.tile([P, KO, SB], bf16, tag="xts")
                nc.sync.dma_start(xts[:], xt_d[i])
                out_sb = xpool.tile([C, SB], bf16, tag="out_sb")
                for st in range(NCH):
                    t = (r0 + st * CHUNK) // BLK  # static task id
                    y = psum.tile([C, CHUNK], f32, tag="y")
                    chunk_group(
                        y,
                        lambda ko: xts[:, ko, st * CHUNK : (st + 1) * CHUNK],
                        t,
                        out_sb[:, st * CHUNK : (st + 1) * CHUNK],
                        CHUNK,
                    )
                # out on the ACT HWDGE ring so it never delays xts loads
                nc.scalar.dma_start(out_d[:, r0 : r0 + SB], out_sb[:])

            # tail: fine-grained stream of the last superblock
            for st in range(NTCH):
                xtl = xpool.tile([P, KO, TCH], bf16, tag="xtl")
                nc.sync.dma_start(xtl[:], xtl_d[st])
                t = (NROW1 + st * TCH) // BLK
                y = psum.tile([C, CHUNK], f32, tag="y")
                out_l = xpool.tile([C, TCH], bf16, tag="out_l")
                chunk_group(y, lambda ko: xtl[:, ko, :], t, out_l[:], TCH)
                nc.scalar.dma_start(
                    out_d[:, NROW1 + st * TCH : NROW1 + (st + 1) * TCH],
                    out_l[:],
                )
    nc.compile()
    return nc


_NC = None


def _get_nc():
    global _NC
    if _NC is None:
        _NC = _build()
    return _NC


def kernel(x, task_labels, W, b):
    global LAST_RESULTS
    x = np.asarray(x)
    if x.dtype != np.float32:
        x = x.astype(np.float32)
    labels = np.asarray(task_labels).astype(np.int64)
    W32 = np.asarray(W)
    if W32.dtype != np.float32:
        W32 = W32.astype(np.float32)
    b32 = np.asarray(b)
    if b32.dtype != np.float32:
        b32 = b32.astype(np.float32)

    wall = np.ascontiguousarray(
        W32.reshape(T, C, KO, P).transpose(3, 0, 2, 1)
    ).astype(ml_dtypes.bfloat16)

    in_maps = []
    placements = []
    for c in range(NCORES):
        lab = labels[c * N : (c + 1) * N]
        xs16 = x[c * N : (c + 1) * N].astype(ml_dtypes.bfloat16)
        slot_to_row = np.full(N, -1, np.int64)
        overflow = []
        for t in range(T):
            idx = np.nonzero(lab == t)[0]
            n_place = min(len(idx), BLK)
            slot_to_row[t * BLK : t * BLK + n_place] = idx[:n_place]
            if len(idx) > BLK:
                overflow.append(idx[BLK:])
        placed = slot_to_row >= 0
        xb = np.zeros((N, D), ml_dtypes.bfloat16)
        xb[placed] = xs16[slot_to_row[placed]]
        # xt[sb, ki, ko, r] = xb[sb*SB + r, ko*P + ki]
        xtf = np.ascontiguousarray(
            xb[:ROW0].reshape(NCH, CHUNK, KO, P).transpose(0, 3, 2, 1)
        )
        xt = np.ascontiguousarray(
            xb[ROW0:NROW1].reshape(NMID, SB, KO, P).transpose(0, 3, 2, 1)
        )
        xtl = np.ascontiguousarray(
            xb[NROW1:].reshape(NTCH, TCH, KO, P).transpose(0, 3, 2, 1)
        )
        in_maps.append({"xt": xt, "xtf": xtf, "xtl": xtl, "wall": wall})
        placements.append(
            (
                slot_to_row,
                placed,
                np.concatenate(overflow) if overflow else np.empty(0, np.int64),
            )
        )

    nc = _get_nc()
    res = bass_utils.run_bass_kernel_spmd(
        nc, in_maps, core_ids=list(range(NCORES)), trace=TRACE
    )
    LAST_RESULTS = res

    out = np.empty((B, C), np.float32)
    for c in range(NCORES):
        dev = np.asarray(res.results[c]["out"]).astype(np.float32).T  # [N, C]
        slot_to_row, placed, overflow = placements[c]
        rows = slot_to_row[placed]
        out[c * N + rows] = dev[placed]
        if len(overflow):
            lab = labels[c * N : (c + 1) * N]
            xs = x[c * N : (c + 1) * N]
            for t in np.unique(lab[overflow]):
                rr = overflow[lab[overflow] == t]
                out[c * N + rr] = xs[rr] @ W32[t].T
    out += b32[labels]
    return out
